# revision 14
# baseline (speedup 1.0000x reference)
"""Trainium2 Bass kernel for nn_BCELoss_64330020159675 (segment_reduce BCE loss).

Data-parallel over batch across 8 NeuronCores, fp8 (e4m3) DoubleRow matmuls.
Host prep: inputs are shipped bf16 and partition-major ([128, chunk, free])
so each DMA descriptor covers an 8-16KB contiguous run (4x fewer descriptors
and half the bytes of the naive f32 row-major layout); the onehot matrix is
precomputed on host in f8e4; counts/scales also host-side (np.bincount).

  phase A: z_i = emb_i / ||emb_i|| cast to f8e4 (DVE), row norms via ACT
    Square+accum; segT[d, c] = sum_b z_i[b, d]*onehot[b, c] with DoubleRow
    PE matmuls, pair-outer in two d-waves of 8 PSUM banks so the PE chases
    the input DMAs.
  One f8e4 AllReduce of segT [D, C] (element-rate-bound, so f8 ~= bf16 cost,
  but f8 needs no post-collective casts).
  phase B (hidden under the collective): column norms of emb_jT via ACT
    Square + PE ones-matmul rows, rsqrt chain on [1,BL], PE k=1 fp32
    broadcast, z_jT = emb_jT * inv in f8e4.
  phase C: Q[c, b] = sum_d segT[d, c]*z_jT[d, b] (DoubleRow matmuls);
    blocks 0-1 run first with held Sqrts (PE warm-up) while |seg_c|^2 rows
    (ones-matmuls) + PE transposes build the Sqrt scale/bias columns;
    diag term via fused DVE scalar_tensor_tensor; softplus sum via Exp
    (bias=2, scale=-1) -> bf16 (1+e) product tree on DVE -> single Ln+accum.
  Host: loss = (sum_cores(out) - 2B) / (B*C).

Identity used: BCEWithLogits elementwise loss = softplus(sim) - match * sim,
and sum(match * sim) = 2*B - sum_b r[b, label_b].
"""
import numpy as np
import ml_dtypes

import concourse.bacc as bacc
import concourse.mybir as mybir
import concourse.tile as tile
from concourse import bass_utils

B = 8192
D = 1024
C = 1024
N_CORES = 8
BL = B // N_CORES          # 1024 rows per core
P = 128                    # partitions
NB = BL // P               # 8 batch chunks per core
ND = D // P                # 8 d chunks
NCC = C // P               # 8 class chunks (partition-major)
NPAIR = NB // 2            # 4 DoubleRow chunk pairs
NBF = BL // 512            # 2 batch free-dim chunks
NCF = C // 512             # 2 class free-dim chunks
NBLK = NCC * NBF           # 16 sim blocks
NHELD = 2                  # sim blocks run before bias prep (PE warm-up)
EPS = 1e-12

F32 = mybir.dt.float32
BF16 = mybir.dt.bfloat16
F8 = mybir.dt.float8e4
AF = mybir.ActivationFunctionType
ALU = mybir.AluOpType
AX = mybir.AxisListType
DR = mybir.MatmulPerfMode.DoubleRow

NP_BF16 = ml_dtypes.bfloat16
NP_F8 = ml_dtypes.float8_e4m3

_NC_CACHE = {}


def build_nc():
    if "nc" in _NC_CACHE:
        return _NC_CACHE["nc"]

    nc = bacc.Bacc(
        "TRN2", target_bir_lowering=False, debug=False, num_devices=N_CORES
    )
    # partition-major bf16 inputs: [p, chunk, x] = orig[chunk*128 + p, x]
    emb_ib = nc.dram_tensor("emb_ib", [P, NB, D], BF16, kind="ExternalInput")
    emb_jTb = nc.dram_tensor("emb_jTb", [P, ND, BL], BF16, kind="ExternalInput")
    oh_in = nc.dram_tensor("oh_in", [P, NB, C], F8, kind="ExternalInput")
    # aux columns: 8:16 ccol, 16:24 scale_col(-2/cnt), 24:32 ic2(1/cnt^2)
    aux = nc.dram_tensor("aux", [P, 32], F32, kind="ExternalInput")
    lab_row = nc.dram_tensor("lab_row", [1, BL], F32, kind="ExternalInput")
    out_partial = nc.dram_tensor("out_partial", [1, 1], F32, kind="ExternalOutput")

    with tile.TileContext(nc) as tc:
        with (
            tc.tile_pool(name="dram", bufs=1, space="DRAM") as dram,
            tc.tile_pool(name="const", bufs=1) as constp,
            tc.tile_pool(name="big", bufs=1) as big,
            tc.tile_pool(name="work", bufs=2) as work,
            tc.tile_pool(name="dump", bufs=1) as dump,
        ):
            cc_in = dram.tile([P, ND, C], F8)
            cc_out = dram.tile([P, ND, C], F8, addr_space="Shared")

            # ---------------- constants / aux ----------------
            aux_t = constp.tile([P, 32], F32)
            nc.sync.dma_start(aux_t[:], aux[:])
            lab_row_t = constp.tile([1, BL], F32)
            nc.sync.dma_start(lab_row_t[:], lab_row[:])
            ones_col_bf = constp.tile([P, 1], BF16)
            nc.vector.memset(ones_col_bf[:], 1.0)
            ones_col_f = constp.tile([P, 1], F32)
            nc.vector.memset(ones_col_f[:], 1.0)
            ones_row_f = constp.tile([1, P], F32)
            nc.vector.memset(ones_row_f[:], 1.0)
            ident1 = constp.tile([1, 1], F32)
            nc.vector.memset(ident1[:], 1.0)
            two_col = constp.tile([P, 1], F32)
            nc.vector.memset(two_col[:], 2.0)

            # ---------------- phase A ----------------
            e_all = big.tile([P, NB, D], BF16, name="e_all")
            oh_all = big.tile([P, NB, C], F8, name="oh_all")
            z_all = big.tile([P, NB, D], F8, name="z_all")
            embT = big.tile([P, ND, BL], BF16, name="embT")
            zjt = big.tile([P, ND, BL], F8, name="zjt")
            sq_dump = dump.tile([P, D], F32, name="sq_dump")

            # input DMAs: pair-granular, spread over the three rings so the
            # critical loads never queue behind non-critical ones
            nc.sync.dma_start(e_all[:, 0:2, :], emb_ib[:, 0:2, :])
            nc.scalar.dma_start(e_all[:, 2:4, :], emb_ib[:, 2:4, :])
            nc.gpsimd.dma_start(e_all[:, 4:6, :], emb_ib[:, 4:6, :])
            nc.sync.dma_start(e_all[:, 6:8, :], emb_ib[:, 6:8, :])
            nc.scalar.dma_start(oh_all[:, 0:4, :], oh_in[:, 0:4, :])
            nc.gpsimd.dma_start(oh_all[:, 4:8, :], oh_in[:, 4:8, :])
            # emb_jT rides the same rings strictly after the critical loads
            nc.sync.dma_start(embT[:, 0:4, :], emb_jTb[:, 0:4, :])
            nc.gpsimd.dma_start(embT[:, 4:8, :], emb_jTb[:, 4:8, :])

            for b in range(NB):
                eb = e_all[:, b, :]
                ss = work.tile([P, 1], F32, tag="ss")
                nc.scalar.activation(sq_dump[:], eb, AF.Square, accum_out=ss[:])
                nrm = work.tile([P, 1], F32, tag="nrm")
                nc.scalar.activation(nrm[:], ss[:], AF.Sqrt)
                nc.vector.tensor_scalar(nrm[:], nrm[:], EPS, None, ALU.max)
                inv = work.tile([P, 1], F32, tag="inv")
                nc.vector.reciprocal(inv[:], nrm[:])
                nc.vector.tensor_scalar(z_all[:, b, :], eb, inv[:], None, ALU.mult)

            # seg matmuls: DoubleRow, pair-outer in two d-waves so the PE
            # chases the chunk DMAs
            with tc.tile_pool(name="psA", bufs=8, space="PSUM") as psA:
                for wave in range(2):
                    gps = [
                        psA.tile([P, 512], F32, tag="seg", name=f"sg{wave}{g}")
                        for g in range(8)
                    ]
                    for pb in range(NPAIR):
                        for g in range(8):
                            d = wave * 4 + g // 2
                            cf = g % 2
                            nc.tensor.matmul(
                                gps[g][:],
                                z_all[:, 2 * pb : 2 * pb + 2, d * P : (d + 1) * P],
                                oh_all[:, 2 * pb : 2 * pb + 2, cf * 512 : (cf + 1) * 512],
                                start=(pb == 0),
                                stop=(pb == NPAIR - 1),
                                perf_mode=DR,
                            )
                    for g in range(8):
                        d = wave * 4 + g // 2
                        cf = g % 2
                        so = work.tile([P, 512], F8, tag="segout", bufs=4)
                        if g % 2 == 0:
                            nc.scalar.copy(so[:], gps[g][:])
                        else:
                            nc.vector.tensor_copy(so[:], gps[g][:])
                        eng = nc.sync if g % 2 == 0 else nc.scalar
                        eng.dma_start(
                            cc_in[:, d, cf * 512 : (cf + 1) * 512], so[:]
                        )

            # ---------------- phase B compute (overlaps collective) ----
            with (
                tc.tile_pool(name="psN", bufs=2, space="PSUM") as psN,
                tc.tile_pool(name="psBc", bufs=2, space="PSUM") as psBc,
            ):
                psn = [
                    psN.tile([1, 512], F32, tag="nrm2", name=f"psn{i}")
                    for i in range(NBF)
                ]
                for d in range(ND):
                    sq2 = work.tile([P, BL], BF16, tag="sqB", bufs=3)
                    nc.scalar.activation(sq2[:], embT[:, d, :], AF.Square)
                    for cf in range(NBF):
                        nc.tensor.matmul(
                            psn[cf][:],
                            ones_col_bf[:],
                            sq2[:, cf * 512 : (cf + 1) * 512],
                            start=(d == 0),
                            stop=(d == ND - 1),
                        )
                nrmj = constp.tile([1, BL], F32)
                for cf in range(NBF):
                    nc.scalar.activation(
                        nrmj[0:1, cf * 512 : (cf + 1) * 512], psn[cf][:], AF.Sqrt
                    )
                nc.vector.tensor_scalar(nrmj[:], nrmj[:], EPS, None, ALU.max)
                invj = constp.tile([1, BL], F32)
                nc.vector.reciprocal(invj[:], nrmj[:])
                # broadcast invj across partitions via k=1 fp32 matmul
                invb = [
                    psBc.tile([P, 512], F32, tag="invb", name=f"invb{i}")
                    for i in range(NBF)
                ]
                for cf in range(NBF):
                    nc.tensor.matmul(
                        invb[cf][:],
                        ones_row_f[:],
                        invj[0:1, cf * 512 : (cf + 1) * 512],
                        start=True,
                        stop=True,
                    )
                for d in range(ND):
                    for cf in range(NBF):
                        nc.vector.tensor_tensor(
                            zjt[:, d, cf * 512 : (cf + 1) * 512],
                            embT[:, d, cf * 512 : (cf + 1) * 512],
                            invb[cf][:],
                            ALU.mult,
                        )

            # lab_bc broadcast (needed in phase C only)
            lab_bc = constp.tile([P, BL], F32)
            with tc.tile_pool(name="psL", bufs=2, space="PSUM") as psL:
                for cf in range(NBF):
                    pl = psL.tile([P, 512], F32, tag="labbc")
                    nc.tensor.matmul(
                        pl[:],
                        ones_row_f[:],
                        lab_row_t[0:1, cf * 512 : (cf + 1) * 512],
                        start=True,
                        stop=True,
                    )
                    nc.scalar.copy(lab_bc[:, cf * 512 : (cf + 1) * 512], pl[:])

            nc.gpsimd.collective_compute(
                "AllReduce",
                ALU.add,
                replica_groups=[list(range(N_CORES))],
                ins=[cc_in[:].opt()],
                outs=[cc_out[:].opt()],
            )

            # ---------------- phase C ----------------
            with tc.tile_pool(name="phC", bufs=1) as pc:
                segf8 = big.tile([P, ND, C], F8, name="segf8")
                for p4 in range(NPAIR):
                    eng = (nc.sync, nc.scalar, nc.gpsimd, nc.sync)[p4]
                    eng.dma_start(
                        segf8[:, 2 * p4 : 2 * p4 + 2, :],
                        cc_out[:, 2 * p4 : 2 * p4 + 2, :],
                    )
                # squares for |seg_c|^2 (same f8 values the Q matmul uses)
                sqC = []
                for d in range(ND):
                    sq = work.tile([P, C], BF16, tag="sqC", bufs=8, name=f"sqc{d}")
                    s8 = segf8[:, d, :]
                    nc.vector.tensor_tensor(sq[:], s8, s8, ALU.mult)
                    sqC.append(sq)

                ssq_row = constp.tile([1, C], F32)
                ssq_col = constp.tile([P, NCC], F32)
                bias_col = constp.tile([P, NCC], F32)
                st = constp.tile([P, NBLK + 1], F32)
                sp_dump = dump.tile([P, 512], F32, name="sp_dump")
                prod_dump = dump.tile([P, 512], F32, name="prod_dump")
                r_all = [pc.tile([P, 512], F32, name=f"r{k}") for k in range(NBLK)]
                ex_all = [pc.tile([P, 512], BF16, name=f"ex{k}") for k in range(NBLK)]

                def emit_sqrt_stt(blk, cc, bf, ps):
                    nc.scalar.activation(
                        r_all[blk][:],
                        ps[:],
                        AF.Sqrt,
                        bias=bias_col[:, cc : cc + 1],
                        scale=aux_t[:, 16 + cc : 17 + cc],
                    )
                    nc.vector.scalar_tensor_tensor(
                        prod_dump[:],
                        lab_bc[:, bf * 512 : (bf + 1) * 512],
                        aux_t[:, 8 + cc : 9 + cc],
                        r_all[blk][:],
                        op0=ALU.is_equal,
                        op1=ALU.mult,
                        accum_out=st[:, blk : blk + 1],
                    )

                with (
                    tc.tile_pool(name="psSim", bufs=4, space="PSUM") as psSim,
                    tc.tile_pool(name="psq", bufs=2, space="PSUM") as psq,
                    tc.tile_pool(name="psT", bufs=2, space="PSUM") as psT,
                ):
                    def emit_block_mms(blk):
                        cc, bf = blk // NBF, blk % NBF
                        ps = psSim.tile([P, 512], F32, tag="sim", name=f"sim{blk}")
                        for pb in range(NPAIR):
                            nc.tensor.matmul(
                                ps[:],
                                segf8[:, 2 * pb : 2 * pb + 2, cc * P : (cc + 1) * P],
                                zjt[:, 2 * pb : 2 * pb + 2, bf * 512 : (bf + 1) * 512],
                                start=(pb == 0),
                                stop=(pb == NPAIR - 1),
                                perf_mode=DR,
                            )
                        return cc, bf, ps

                    # held blocks double as PE pstate warm-up under bias prep
                    held = [(blk, *emit_block_mms(blk)) for blk in range(NHELD)]

                    # bias prep: ssq rows, transposes, folds
                    pq = [
                        psq.tile([1, 512], F32, tag="ssq", name=f"pq{i}")
                        for i in range(NCF)
                    ]
                    for cf in range(NCF):
                        for d in range(ND):
                            nc.tensor.matmul(
                                pq[cf][:],
                                ones_col_bf[:],
                                sqC[d][:, cf * 512 : (cf + 1) * 512],
                                start=(d == 0),
                                stop=(d == ND - 1),
                            )
                        nc.vector.tensor_copy(
                            ssq_row[0:1, cf * 512 : (cf + 1) * 512], pq[cf][:]
                        )
                    for cc in range(NCC):
                        pt = psT.tile([P, 1], F32, tag="tr")
                        nc.tensor.transpose(
                            pt[:], ssq_row[0:1, cc * P : (cc + 1) * P], ident1[:]
                        )
                        nc.vector.tensor_copy(ssq_col[:, cc : cc + 1], pt[:])
                    nc.vector.tensor_tensor(
                        bias_col[:], ssq_col[:], aux_t[:, 24:32], ALU.mult
                    )
                    nc.vector.tensor_scalar(
                        bias_col[:], bias_col[:], 1.0, None, ALU.add
                    )

                    def emit_exp(blk):
                        # softplus: exp (bias=2, scale=-1) then +1 on DVE
                        nc.scalar.activation(
                            ex_all[blk][:], r_all[blk][:], AF.Exp,
                            bias=two_col[:], scale=-1.0,
                        )
                        nc.vector.tensor_scalar(
                            ex_all[blk][:], ex_all[blk][:], 1.0, None, ALU.add
                        )

                    for blk, cc, bf, ps in held:
                        emit_sqrt_stt(blk, cc, bf, ps)
                    for blk in range(NHELD, NBLK):
                        cc, bf, ps = emit_block_mms(blk)
                        emit_sqrt_stt(blk, cc, bf, ps)
                    for k in range(NBLK):
                        emit_exp(k)

                step = 1
                while step < NBLK:
                    for k in range(0, NBLK, 2 * step):
                        nc.vector.tensor_tensor(
                            ex_all[k][:], ex_all[k][:], ex_all[k + step][:], ALU.mult
                        )
                    step *= 2
                nc.scalar.activation(
                    sp_dump[:],
                    ex_all[0][:],
                    AF.Ln,
                    accum_out=st[:, NBLK : NBLK + 1],
                )

                with tc.tile_pool(name="psFin", bufs=1, space="PSUM") as psFin:
                    pf = psFin.tile([1, NBLK + 1], F32, tag="fin")
                    nc.tensor.matmul(pf[:], ones_col_f[:], st[:], start=True, stop=True)
                    fin_row = constp.tile([1, NBLK + 1], F32)
                    nc.vector.tensor_copy(fin_row[:], pf[:])
                    tot = constp.tile([1, 1], F32)
                    nc.vector.tensor_reduce(tot[:], fin_row[:], axis=AX.X, op=ALU.add)
                    nc.sync.dma_start(out_partial[0:1, 0:1], tot[:])

    nc.compile()
    _NC_CACHE["nc"] = nc
    return nc


def make_in_maps(emb_i, emb_j, labels):
    emb_i = np.asarray(emb_i, dtype=np.float32)
    emb_j = np.asarray(emb_j, dtype=np.float32)
    lab = np.asarray(labels).astype(np.int64)
    labf = lab.astype(np.float32)
    counts = np.bincount(lab, minlength=C).astype(np.float32)  # all >= 1
    ccol = (
        np.arange(P, dtype=np.float32)[:, None]
        + P * np.arange(NCC, dtype=np.float32)[None, :]
    )
    cnt_col = counts.reshape(NCC, P).T          # [P, NCC], class cc*128+p
    scale_col = -2.0 / cnt_col
    ic2_col = 1.0 / (cnt_col * cnt_col)
    in_maps = []
    for k in range(N_CORES):
        sl = slice(k * BL, (k + 1) * BL)
        lab_k = lab[sl]
        # partition-major bf16 inputs
        eib = np.ascontiguousarray(
            emb_i[sl].astype(NP_BF16).reshape(NB, P, D).transpose(1, 0, 2)
        )
        ejtb = np.ascontiguousarray(
            emb_j[sl].T.astype(NP_BF16).reshape(ND, P, BL).transpose(1, 0, 2)
        )
        oh = np.zeros((BL, C), dtype=NP_F8)
        oh[np.arange(BL), lab_k] = NP_F8(1.0)
        ohr = np.ascontiguousarray(oh.reshape(NB, P, C).transpose(1, 0, 2))
        aux = np.zeros((P, 32), np.float32)
        aux[:, 8 : 8 + NCC] = ccol
        aux[:, 16 : 16 + NCC] = scale_col
        aux[:, 24 : 24 + NCC] = ic2_col
        in_maps.append(
            {
                "emb_ib": eib,
                "emb_jTb": ejtb,
                "oh_in": ohr,
                "aux": np.ascontiguousarray(aux),
                "lab_row": np.ascontiguousarray(labf[sl][None, :]),
            }
        )
    return in_maps


def combine_partials(results):
    tot = 0.0
    for k in range(N_CORES):
        p = np.asarray(results[k]["out_partial"], dtype=np.float64)
        tot += p[0, 0]
    loss = (tot - 2.0 * B) / (B * C)
    return np.asarray(np.float32(loss))


def run(emb_i, emb_j, labels, **run_kwargs):
    nc = build_nc()
    in_maps = make_in_maps(emb_i, emb_j, labels)
    res = bass_utils.run_bass_kernel_spmd(
        nc, in_maps, core_ids=list(range(N_CORES)), **run_kwargs
    )
    return combine_partials(res.results), res


def kernel(emb_i, emb_j, labels):
    loss, _ = run(emb_i, emb_j, labels)
    return loss


# revision 15
# speedup vs baseline: 1.1164x; 1.1164x over previous
"""Trainium2 Bass kernel for nn_BCELoss_64330020159675 (segment_reduce BCE loss).

Data-parallel over batch across 8 NeuronCores, fp8 (e4m3) DoubleRow matmuls.
Host prep: inputs are shipped bf16 and partition-major ([128, chunk, free])
so each DMA descriptor covers an 8-16KB contiguous run (4x fewer descriptors
and half the bytes of the naive f32 row-major layout); the onehot matrix is
precomputed on host in f8e4; counts/scales also host-side (np.bincount).

  phase A: z_i = emb_i / ||emb_i|| cast to f8e4 (DVE), row norms via ACT
    Square+accum; segT[d, c] = sum_b z_i[b, d]*onehot[b, c] with DoubleRow
    PE matmuls, pair-outer in two d-waves of 8 PSUM banks so the PE chases
    the input DMAs.
  One f8e4 AllReduce of segT [D, C] (element-rate-bound, so f8 ~= bf16 cost,
  but f8 needs no post-collective casts).
  phase B (hidden under the collective): column norms of emb_jT via ACT
    Square + PE ones-matmul rows, rsqrt chain on [1,BL], PE k=1 fp32
    broadcast, z_jT = emb_jT * inv in f8e4.
  phase C: Q[c, b] = sum_d segT[d, c]*z_jT[d, b] (DoubleRow matmuls);
    blocks 0-1 run first with held Sqrts (PE warm-up) while |seg_c|^2 rows
    (ones-matmuls) + PE transposes build the Sqrt scale/bias columns;
    diag term via fused DVE scalar_tensor_tensor; softplus sum via Exp
    (bias=2, scale=-1) -> bf16 (1+e) product tree on DVE -> single Ln+accum.
  Host: loss = (sum_cores(out) - 2B) / (B*C).

Identity used: BCEWithLogits elementwise loss = softplus(sim) - match * sim,
and sum(match * sim) = 2*B - sum_b r[b, label_b].
"""
import numpy as np
import ml_dtypes

import concourse.bacc as bacc
import concourse.mybir as mybir
import concourse.tile as tile
from concourse import bass_utils

B = 8192
D = 1024
C = 1024
N_CORES = 8
BL = B // N_CORES          # 1024 rows per core
P = 128                    # partitions
NB = BL // P               # 8 batch chunks per core
ND = D // P                # 8 d chunks
NCC = C // P               # 8 class chunks (partition-major)
NPAIR = NB // 2            # 4 DoubleRow chunk pairs
NBF = BL // 512            # 2 batch free-dim chunks
NCF = C // 512             # 2 class free-dim chunks
NBLK = NCC * NBF           # 16 sim blocks
NHELD = 2                  # sim blocks run before bias prep (PE warm-up)
EPS = 1e-12

F32 = mybir.dt.float32
BF16 = mybir.dt.bfloat16
F8 = mybir.dt.float8e4
AF = mybir.ActivationFunctionType
ALU = mybir.AluOpType
AX = mybir.AxisListType
DR = mybir.MatmulPerfMode.DoubleRow

NP_BF16 = ml_dtypes.bfloat16
NP_F8 = ml_dtypes.float8_e4m3

_NC_CACHE = {}


def build_nc():
    if "nc" in _NC_CACHE:
        return _NC_CACHE["nc"]

    nc = bacc.Bacc(
        "TRN2", target_bir_lowering=False, debug=False, num_devices=N_CORES
    )
    # partition-major bf16 inputs: [p, chunk, x] = orig[chunk*128 + p, x]
    emb_ib = nc.dram_tensor("emb_ib", [P, NB, D], BF16, kind="ExternalInput")
    emb_jTb = nc.dram_tensor("emb_jTb", [P, ND, BL], BF16, kind="ExternalInput")
    oh_in = nc.dram_tensor("oh_in", [P, NB, C], F8, kind="ExternalInput")
    # aux columns: 8:16 ccol, 16:24 scale_col(-2/cnt), 24:32 ic2(1/cnt^2)
    aux = nc.dram_tensor("aux", [P, 32], F32, kind="ExternalInput")
    lab_row = nc.dram_tensor("lab_row", [1, BL], F32, kind="ExternalInput")
    out_partial = nc.dram_tensor("out_partial", [1, 1], F32, kind="ExternalOutput")

    with tile.TileContext(nc) as tc:
        with (
            tc.tile_pool(name="dram", bufs=1, space="DRAM") as dram,
            tc.tile_pool(name="const", bufs=1) as constp,
            tc.tile_pool(name="big", bufs=1) as big,
            tc.tile_pool(name="work", bufs=2) as work,
            tc.tile_pool(name="dump", bufs=1) as dump,
        ):
            cc_in = dram.tile([P, ND, C], F8)
            cc_out = dram.tile([P, ND, C], F8, addr_space="Shared")

            # ---------------- constants / aux ----------------
            aux_t = constp.tile([P, 32], F32)
            nc.sync.dma_start(aux_t[:], aux[:])
            lab_row_t = constp.tile([1, BL], F32)
            nc.sync.dma_start(lab_row_t[:], lab_row[:])
            ones_col_bf = constp.tile([P, 1], BF16)
            nc.vector.memset(ones_col_bf[:], 1.0)
            ones_col_f = constp.tile([P, 1], F32)
            nc.vector.memset(ones_col_f[:], 1.0)
            ones_row_f = constp.tile([1, P], F32)
            nc.vector.memset(ones_row_f[:], 1.0)
            ident1 = constp.tile([1, 1], F32)
            nc.vector.memset(ident1[:], 1.0)
            two_col = constp.tile([P, 1], F32)
            nc.vector.memset(two_col[:], 2.0)

            # ---------------- phase A ----------------
            e_all = big.tile([P, NB, D], BF16, name="e_all")
            oh_all = big.tile([P, NB, C], F8, name="oh_all")
            z_all = big.tile([P, NB, D], F8, name="z_all")
            embT = big.tile([P, ND, BL], BF16, name="embT")
            zjt = big.tile([P, ND, BL], F8, name="zjt")
            sq_dump = dump.tile([P, D], F32, name="sq_dump")

            # input DMAs: pair-granular, spread over the three rings so the
            # critical loads never queue behind non-critical ones
            nc.sync.dma_start(e_all[:], emb_ib[:])
            nc.scalar.dma_start(oh_all[:], oh_in[:])
            # emb_jT rides the rings strictly after the critical loads
            nc.gpsimd.dma_start(embT[:], emb_jTb[:])

            for b in range(NB):
                eb = e_all[:, b, :]
                ss = work.tile([P, 1], F32, tag="ss")
                nc.scalar.activation(sq_dump[:], eb, AF.Square, accum_out=ss[:])
                nrm = work.tile([P, 1], F32, tag="nrm")
                nc.scalar.activation(nrm[:], ss[:], AF.Sqrt)
                nc.vector.tensor_scalar(nrm[:], nrm[:], EPS, None, ALU.max)
                inv = work.tile([P, 1], F32, tag="inv")
                nc.vector.reciprocal(inv[:], nrm[:])
                nc.vector.tensor_scalar(z_all[:, b, :], eb, inv[:], None, ALU.mult)

            # seg matmuls: DoubleRow, pair-outer in two d-waves so the PE
            # chases the chunk DMAs
            with tc.tile_pool(name="psA", bufs=8, space="PSUM") as psA:
                for wave in range(2):
                    gps = [
                        psA.tile([P, 512], F32, tag="seg", name=f"sg{wave}{g}")
                        for g in range(8)
                    ]
                    for pb in range(NPAIR):
                        for g in range(8):
                            d = wave * 4 + g // 2
                            cf = g % 2
                            nc.tensor.matmul(
                                gps[g][:],
                                z_all[:, 2 * pb : 2 * pb + 2, d * P : (d + 1) * P],
                                oh_all[:, 2 * pb : 2 * pb + 2, cf * 512 : (cf + 1) * 512],
                                start=(pb == 0),
                                stop=(pb == NPAIR - 1),
                                perf_mode=DR,
                            )
                    for dd in range(4):
                        d = wave * 4 + dd
                        so = work.tile([P, C], F8, tag="segout", bufs=3)
                        nc.scalar.copy(so[:, 0:512], gps[2 * dd][:])
                        nc.vector.tensor_copy(so[:, 512:1024], gps[2 * dd + 1][:])
                        eng = nc.sync if dd % 2 == 0 else nc.scalar
                        eng.dma_start(cc_in[:, d, :], so[:])

            # ---------------- phase B compute (overlaps collective) ----
            with (
                tc.tile_pool(name="psN", bufs=2, space="PSUM") as psN,
                tc.tile_pool(name="psBc", bufs=2, space="PSUM") as psBc,
            ):
                psn = [
                    psN.tile([1, 512], F32, tag="nrm2", name=f"psn{i}")
                    for i in range(NBF)
                ]
                for d in range(ND):
                    sq2 = work.tile([P, BL], BF16, tag="sqB", bufs=3)
                    nc.scalar.activation(sq2[:], embT[:, d, :], AF.Square)
                    for cf in range(NBF):
                        nc.tensor.matmul(
                            psn[cf][:],
                            ones_col_bf[:],
                            sq2[:, cf * 512 : (cf + 1) * 512],
                            start=(d == 0),
                            stop=(d == ND - 1),
                        )
                nrmj = constp.tile([1, BL], F32)
                for cf in range(NBF):
                    nc.scalar.activation(
                        nrmj[0:1, cf * 512 : (cf + 1) * 512], psn[cf][:], AF.Sqrt
                    )
                nc.vector.tensor_scalar(nrmj[:], nrmj[:], EPS, None, ALU.max)
                invj = constp.tile([1, BL], F32)
                nc.vector.reciprocal(invj[:], nrmj[:])
                # broadcast invj across partitions via k=1 fp32 matmul
                invb = [
                    psBc.tile([P, 512], F32, tag="invb", name=f"invb{i}")
                    for i in range(NBF)
                ]
                for cf in range(NBF):
                    nc.tensor.matmul(
                        invb[cf][:],
                        ones_row_f[:],
                        invj[0:1, cf * 512 : (cf + 1) * 512],
                        start=True,
                        stop=True,
                    )
                for d in range(ND):
                    for cf in range(NBF):
                        nc.vector.tensor_tensor(
                            zjt[:, d, cf * 512 : (cf + 1) * 512],
                            embT[:, d, cf * 512 : (cf + 1) * 512],
                            invb[cf][:],
                            ALU.mult,
                        )

            # lab_bc broadcast (needed in phase C only)
            lab_bc = constp.tile([P, BL], F32)
            with tc.tile_pool(name="psL", bufs=2, space="PSUM") as psL:
                for cf in range(NBF):
                    pl = psL.tile([P, 512], F32, tag="labbc")
                    nc.tensor.matmul(
                        pl[:],
                        ones_row_f[:],
                        lab_row_t[0:1, cf * 512 : (cf + 1) * 512],
                        start=True,
                        stop=True,
                    )
                    nc.scalar.copy(lab_bc[:, cf * 512 : (cf + 1) * 512], pl[:])

            nc.gpsimd.collective_compute(
                "AllReduce",
                ALU.add,
                replica_groups=[list(range(N_CORES))],
                ins=[cc_in[:].opt()],
                outs=[cc_out[:].opt()],
            )

            # ---------------- phase C ----------------
            with tc.tile_pool(name="phC", bufs=1) as pc:
                segf8 = big.tile([P, ND, C], F8, name="segf8")
                for p4 in range(NPAIR):
                    eng = (nc.sync, nc.scalar, nc.gpsimd, nc.sync)[p4]
                    eng.dma_start(
                        segf8[:, 2 * p4 : 2 * p4 + 2, :],
                        cc_out[:, 2 * p4 : 2 * p4 + 2, :],
                    )
                # squares for |seg_c|^2 (same f8 values the Q matmul uses)
                sqC = []
                for d in range(ND):
                    sq = work.tile([P, C], BF16, tag="sqC", bufs=8, name=f"sqc{d}")
                    s8 = segf8[:, d, :]
                    nc.vector.tensor_tensor(sq[:], s8, s8, ALU.mult)
                    sqC.append(sq)

                ssq_row = constp.tile([1, C], F32)
                ssq_col = constp.tile([P, NCC], F32)
                bias_col = constp.tile([P, NCC], F32)
                st = constp.tile([P, NBLK + 1], F32)
                sp_dump = dump.tile([P, 512], F32, name="sp_dump")
                prod_dump = dump.tile([P, 512], F32, name="prod_dump")
                r_all = [pc.tile([P, 512], F32, name=f"r{k}") for k in range(NBLK)]
                ex_all = [pc.tile([P, 512], BF16, name=f"ex{k}") for k in range(NBLK)]

                def emit_sqrt_stt(blk, cc, bf, ps):
                    nc.scalar.activation(
                        r_all[blk][:],
                        ps[:],
                        AF.Sqrt,
                        bias=bias_col[:, cc : cc + 1],
                        scale=aux_t[:, 16 + cc : 17 + cc],
                    )
                    nc.vector.scalar_tensor_tensor(
                        prod_dump[:],
                        lab_bc[:, bf * 512 : (bf + 1) * 512],
                        aux_t[:, 8 + cc : 9 + cc],
                        r_all[blk][:],
                        op0=ALU.is_equal,
                        op1=ALU.mult,
                        accum_out=st[:, blk : blk + 1],
                    )

                with (
                    tc.tile_pool(name="psSim", bufs=4, space="PSUM") as psSim,
                    tc.tile_pool(name="psq", bufs=2, space="PSUM") as psq,
                    tc.tile_pool(name="psT", bufs=2, space="PSUM") as psT,
                ):
                    def emit_block_mms(blk):
                        cc, bf = blk // NBF, blk % NBF
                        ps = psSim.tile([P, 512], F32, tag="sim", name=f"sim{blk}")
                        for pb in range(NPAIR):
                            nc.tensor.matmul(
                                ps[:],
                                segf8[:, 2 * pb : 2 * pb + 2, cc * P : (cc + 1) * P],
                                zjt[:, 2 * pb : 2 * pb + 2, bf * 512 : (bf + 1) * 512],
                                start=(pb == 0),
                                stop=(pb == NPAIR - 1),
                                perf_mode=DR,
                            )
                        return cc, bf, ps

                    # held blocks double as PE pstate warm-up under bias prep
                    held = [(blk, *emit_block_mms(blk)) for blk in range(NHELD)]

                    # bias prep: ssq rows, transposes, folds
                    pq = [
                        psq.tile([1, 512], F32, tag="ssq", name=f"pq{i}")
                        for i in range(NCF)
                    ]
                    for cf in range(NCF):
                        for d in range(ND):
                            nc.tensor.matmul(
                                pq[cf][:],
                                ones_col_bf[:],
                                sqC[d][:, cf * 512 : (cf + 1) * 512],
                                start=(d == 0),
                                stop=(d == ND - 1),
                            )
                        nc.vector.tensor_copy(
                            ssq_row[0:1, cf * 512 : (cf + 1) * 512], pq[cf][:]
                        )
                    for cc in range(NCC):
                        pt = psT.tile([P, 1], F32, tag="tr")
                        nc.tensor.transpose(
                            pt[:], ssq_row[0:1, cc * P : (cc + 1) * P], ident1[:]
                        )
                        nc.vector.tensor_copy(ssq_col[:, cc : cc + 1], pt[:])
                    nc.vector.tensor_tensor(
                        bias_col[:], ssq_col[:], aux_t[:, 24:32], ALU.mult
                    )
                    nc.vector.tensor_scalar(
                        bias_col[:], bias_col[:], 1.0, None, ALU.add
                    )

                    def emit_exp(blk):
                        # softplus: exp (bias=2, scale=-1) then +1 on DVE
                        nc.scalar.activation(
                            ex_all[blk][:], r_all[blk][:], AF.Exp,
                            bias=two_col[:], scale=-1.0,
                        )
                        nc.vector.tensor_scalar(
                            ex_all[blk][:], ex_all[blk][:], 1.0, None, ALU.add
                        )

                    for blk, cc, bf, ps in held:
                        emit_sqrt_stt(blk, cc, bf, ps)
                    for blk in range(NHELD, NBLK):
                        cc, bf, ps = emit_block_mms(blk)
                        emit_sqrt_stt(blk, cc, bf, ps)
                    for k in range(NBLK):
                        emit_exp(k)

                step = 1
                while step < NBLK:
                    for k in range(0, NBLK, 2 * step):
                        nc.vector.tensor_tensor(
                            ex_all[k][:], ex_all[k][:], ex_all[k + step][:], ALU.mult
                        )
                    step *= 2
                nc.scalar.activation(
                    sp_dump[:],
                    ex_all[0][:],
                    AF.Ln,
                    accum_out=st[:, NBLK : NBLK + 1],
                )

                with tc.tile_pool(name="psFin", bufs=1, space="PSUM") as psFin:
                    pf = psFin.tile([1, NBLK + 1], F32, tag="fin")
                    nc.tensor.matmul(pf[:], ones_col_f[:], st[:], start=True, stop=True)
                    fin_row = constp.tile([1, NBLK + 1], F32)
                    nc.vector.tensor_copy(fin_row[:], pf[:])
                    tot = constp.tile([1, 1], F32)
                    nc.vector.tensor_reduce(tot[:], fin_row[:], axis=AX.X, op=ALU.add)
                    nc.sync.dma_start(out_partial[0:1, 0:1], tot[:])

    nc.compile()
    _NC_CACHE["nc"] = nc
    return nc


def make_in_maps(emb_i, emb_j, labels):
    emb_i = np.asarray(emb_i, dtype=np.float32)
    emb_j = np.asarray(emb_j, dtype=np.float32)
    lab = np.asarray(labels).astype(np.int64)
    labf = lab.astype(np.float32)
    counts = np.bincount(lab, minlength=C).astype(np.float32)  # all >= 1
    ccol = (
        np.arange(P, dtype=np.float32)[:, None]
        + P * np.arange(NCC, dtype=np.float32)[None, :]
    )
    cnt_col = counts.reshape(NCC, P).T          # [P, NCC], class cc*128+p
    scale_col = -2.0 / cnt_col
    ic2_col = 1.0 / (cnt_col * cnt_col)
    in_maps = []
    for k in range(N_CORES):
        sl = slice(k * BL, (k + 1) * BL)
        lab_k = lab[sl]
        # partition-major bf16 inputs
        eib = np.ascontiguousarray(
            emb_i[sl].astype(NP_BF16).reshape(NB, P, D).transpose(1, 0, 2)
        )
        ejtb = np.ascontiguousarray(
            emb_j[sl].T.astype(NP_BF16).reshape(ND, P, BL).transpose(1, 0, 2)
        )
        oh = np.zeros((BL, C), dtype=NP_F8)
        oh[np.arange(BL), lab_k] = NP_F8(1.0)
        ohr = np.ascontiguousarray(oh.reshape(NB, P, C).transpose(1, 0, 2))
        aux = np.zeros((P, 32), np.float32)
        aux[:, 8 : 8 + NCC] = ccol
        aux[:, 16 : 16 + NCC] = scale_col
        aux[:, 24 : 24 + NCC] = ic2_col
        in_maps.append(
            {
                "emb_ib": eib,
                "emb_jTb": ejtb,
                "oh_in": ohr,
                "aux": np.ascontiguousarray(aux),
                "lab_row": np.ascontiguousarray(labf[sl][None, :]),
            }
        )
    return in_maps


def combine_partials(results):
    tot = 0.0
    for k in range(N_CORES):
        p = np.asarray(results[k]["out_partial"], dtype=np.float64)
        tot += p[0, 0]
    loss = (tot - 2.0 * B) / (B * C)
    return np.asarray(np.float32(loss))


def run(emb_i, emb_j, labels, **run_kwargs):
    nc = build_nc()
    in_maps = make_in_maps(emb_i, emb_j, labels)
    res = bass_utils.run_bass_kernel_spmd(
        nc, in_maps, core_ids=list(range(N_CORES)), **run_kwargs
    )
    return combine_partials(res.results), res


def kernel(emb_i, emb_j, labels):
    loss, _ = run(emb_i, emb_j, labels)
    return loss


# revision 16
# speedup vs baseline: 1.1711x; 1.0489x over previous
"""Trainium2 Bass kernel for nn_BCELoss_64330020159675 (segment_reduce BCE loss).

Data-parallel over batch across 8 NeuronCores, fp8 (e4m3) DoubleRow matmuls.
Host prep: inputs are shipped bf16 and partition-major ([128, chunk, free])
so each DMA descriptor covers an 8-16KB contiguous run (4x fewer descriptors
and half the bytes of the naive f32 row-major layout); the onehot matrix is
precomputed on host in f8e4; counts/scales also host-side (np.bincount).

  phase A: z_i = emb_i / ||emb_i|| cast to f8e4 (DVE), row norms via ACT
    Square+accum; segT[d, c] = sum_b z_i[b, d]*onehot[b, c] with DoubleRow
    PE matmuls, pair-outer in two d-waves of 8 PSUM banks so the PE chases
    the input DMAs.
  One f8e4 AllReduce of segT [D, C] (element-rate-bound, so f8 ~= bf16 cost,
  but f8 needs no post-collective casts).
  phase B (hidden under the collective): column norms of emb_jT via ACT
    Square + PE ones-matmul rows, rsqrt chain on [1,BL], PE k=1 fp32
    broadcast, z_jT = emb_jT * inv in f8e4.
  phase C: Q[c, b] = sum_d segT[d, c]*z_jT[d, b] (DoubleRow matmuls);
    blocks 0-1 run first with held Sqrts (PE warm-up) while |seg_c|^2 rows
    (ones-matmuls) + PE transposes build the Sqrt scale/bias columns;
    diag term via fused DVE scalar_tensor_tensor; softplus sum via Exp
    (bias=2, scale=-1) -> bf16 (1+e) product tree on DVE -> single Ln+accum.
  Host: loss = (sum_cores(out) - 2B) / (B*C).

Identity used: BCEWithLogits elementwise loss = softplus(sim) - match * sim,
and sum(match * sim) = 2*B - sum_b r[b, label_b].
"""
import numpy as np
import ml_dtypes

import concourse.bacc as bacc
import concourse.mybir as mybir
import concourse.tile as tile
from concourse import bass_utils

B = 8192
D = 1024
C = 1024
N_CORES = 8
BL = B // N_CORES          # 1024 rows per core
P = 128                    # partitions
NB = BL // P               # 8 batch chunks per core
ND = D // P                # 8 d chunks
NCC = C // P               # 8 class chunks (partition-major)
NPAIR = NB // 2            # 4 DoubleRow chunk pairs
NBF = BL // 512            # 2 batch free-dim chunks
NCF = C // 512             # 2 class free-dim chunks
NBLK = NCC * NBF           # 16 sim blocks
NHELD = 2                  # sim blocks run before bias prep (PE warm-up)
EPS = 1e-12

F32 = mybir.dt.float32
BF16 = mybir.dt.bfloat16
F8 = mybir.dt.float8e4
AF = mybir.ActivationFunctionType
ALU = mybir.AluOpType
AX = mybir.AxisListType
DR = mybir.MatmulPerfMode.DoubleRow

NP_BF16 = ml_dtypes.bfloat16
NP_F8 = ml_dtypes.float8_e4m3

_NC_CACHE = {}


def build_nc():
    if "nc" in _NC_CACHE:
        return _NC_CACHE["nc"]

    nc = bacc.Bacc(
        "TRN2", target_bir_lowering=False, debug=False, num_devices=N_CORES
    )
    # partition-major bf16 inputs: [p, chunk, x] = orig[chunk*128 + p, x]
    emb_ib = nc.dram_tensor("emb_ib", [P, NB, D], BF16, kind="ExternalInput")
    emb_jTb = nc.dram_tensor("emb_jTb", [P, ND, BL], BF16, kind="ExternalInput")
    oh_in = nc.dram_tensor("oh_in", [P, NB, C], F8, kind="ExternalInput")
    # aux columns: 8:16 ccol, 16:24 scale_col(-2/cnt), 24:32 ic2(1/cnt^2)
    aux = nc.dram_tensor("aux", [P, 32], F32, kind="ExternalInput")
    lab_row = nc.dram_tensor("lab_row", [1, BL], F32, kind="ExternalInput")
    out_partial = nc.dram_tensor("out_partial", [1, 1], F32, kind="ExternalOutput")

    with tile.TileContext(nc) as tc:
        with (
            tc.tile_pool(name="dram", bufs=1, space="DRAM") as dram,
            tc.tile_pool(name="const", bufs=1) as constp,
            tc.tile_pool(name="big", bufs=1) as big,
            tc.tile_pool(name="work", bufs=2) as work,
            tc.tile_pool(name="dump", bufs=1) as dump,
        ):
            cc_in = dram.tile([P, ND, C], F8)
            cc_out = dram.tile([P, ND, C], F8, addr_space="Shared")

            # ---------------- constants / aux ----------------
            aux_t = constp.tile([P, 32], F32)
            nc.sync.dma_start(aux_t[:], aux[:])
            lab_row_t = constp.tile([1, BL], F32)
            nc.sync.dma_start(lab_row_t[:], lab_row[:])
            ones_col_bf = constp.tile([P, 1], BF16)
            nc.vector.memset(ones_col_bf[:], 1.0)
            ones_col_f = constp.tile([P, 1], F32)
            nc.vector.memset(ones_col_f[:], 1.0)
            ones_row_f = constp.tile([1, P], F32)
            nc.vector.memset(ones_row_f[:], 1.0)
            ident1 = constp.tile([1, 1], F32)
            nc.vector.memset(ident1[:], 1.0)
            two_col = constp.tile([P, 1], F32)
            nc.vector.memset(two_col[:], 2.0)

            # ---------------- phase A ----------------
            e_all = big.tile([P, NB, D], BF16, name="e_all")
            oh_all = big.tile([P, NB, C], F8, name="oh_all")
            z_all = big.tile([P, NB, D], F8, name="z_all")
            embT = big.tile([P, ND, BL], BF16, name="embT")
            zjt = big.tile([P, ND, BL], F8, name="zjt")
            sq_dump = dump.tile([P, D], F32, name="sq_dump")

            # input DMAs: pair-granular, spread over the three rings so the
            # critical loads never queue behind non-critical ones
            nc.sync.dma_start(e_all[:, 0:4, :], emb_ib[:, 0:4, :])
            nc.scalar.dma_start(e_all[:, 4:8, :], emb_ib[:, 4:8, :])
            nc.gpsimd.dma_start(oh_all[:], oh_in[:])
            # emb_jT rides the ring strictly after the critical loads
            nc.gpsimd.dma_start(embT[:], emb_jTb[:])

            for b in range(NB):
                eb = e_all[:, b, :]
                ss = work.tile([P, 1], F32, tag="ss")
                nc.scalar.activation(sq_dump[:], eb, AF.Square, accum_out=ss[:])
                nrm = work.tile([P, 1], F32, tag="nrm")
                nc.scalar.activation(nrm[:], ss[:], AF.Sqrt)
                nc.vector.tensor_scalar(nrm[:], nrm[:], EPS, None, ALU.max)
                inv = work.tile([P, 1], F32, tag="inv")
                nc.vector.reciprocal(inv[:], nrm[:])
                nc.vector.tensor_scalar(z_all[:, b, :], eb, inv[:], None, ALU.mult)

            # seg matmuls: DoubleRow, pair-outer in two d-waves so the PE
            # chases the chunk DMAs
            with tc.tile_pool(name="psA", bufs=8, space="PSUM") as psA:
                for wave in range(2):
                    gps = [
                        psA.tile([P, 512], F32, tag="seg", name=f"sg{wave}{g}")
                        for g in range(8)
                    ]
                    for pb in range(NPAIR):
                        for g in range(8):
                            d = wave * 4 + g // 2
                            cf = g % 2
                            nc.tensor.matmul(
                                gps[g][:],
                                z_all[:, 2 * pb : 2 * pb + 2, d * P : (d + 1) * P],
                                oh_all[:, 2 * pb : 2 * pb + 2, cf * 512 : (cf + 1) * 512],
                                start=(pb == 0),
                                stop=(pb == NPAIR - 1),
                                perf_mode=DR,
                            )
                    for dd in range(4):
                        d = wave * 4 + dd
                        so = work.tile([P, C], F8, tag="segout", bufs=3)
                        nc.scalar.copy(so[:, 0:512], gps[2 * dd][:])
                        nc.vector.tensor_copy(so[:, 512:1024], gps[2 * dd + 1][:])
                        eng = nc.sync if dd % 2 == 0 else nc.scalar
                        eng.dma_start(cc_in[:, d, :], so[:])

            # ---------------- phase B compute (overlaps collective) ----
            with (
                tc.tile_pool(name="psN", bufs=2, space="PSUM") as psN,
                tc.tile_pool(name="psBc", bufs=2, space="PSUM") as psBc,
            ):
                psn = [
                    psN.tile([1, 512], F32, tag="nrm2", name=f"psn{i}")
                    for i in range(NBF)
                ]
                for d in range(ND):
                    sq2 = work.tile([P, BL], BF16, tag="sqB", bufs=3)
                    nc.scalar.activation(sq2[:], embT[:, d, :], AF.Square)
                    for cf in range(NBF):
                        nc.tensor.matmul(
                            psn[cf][:],
                            ones_col_bf[:],
                            sq2[:, cf * 512 : (cf + 1) * 512],
                            start=(d == 0),
                            stop=(d == ND - 1),
                        )
                nrmj = constp.tile([1, BL], F32)
                for cf in range(NBF):
                    nc.scalar.activation(
                        nrmj[0:1, cf * 512 : (cf + 1) * 512], psn[cf][:], AF.Sqrt
                    )
                nc.vector.tensor_scalar(nrmj[:], nrmj[:], EPS, None, ALU.max)
                invj = constp.tile([1, BL], F32)
                nc.vector.reciprocal(invj[:], nrmj[:])
                # broadcast invj across partitions via k=1 fp32 matmul
                invb = [
                    psBc.tile([P, 512], F32, tag="invb", name=f"invb{i}")
                    for i in range(NBF)
                ]
                for cf in range(NBF):
                    nc.tensor.matmul(
                        invb[cf][:],
                        ones_row_f[:],
                        invj[0:1, cf * 512 : (cf + 1) * 512],
                        start=True,
                        stop=True,
                    )
                for d in range(ND):
                    for cf in range(NBF):
                        nc.vector.tensor_tensor(
                            zjt[:, d, cf * 512 : (cf + 1) * 512],
                            embT[:, d, cf * 512 : (cf + 1) * 512],
                            invb[cf][:],
                            ALU.mult,
                        )

            # lab_bc broadcast (needed in phase C only)
            lab_bc = constp.tile([P, BL], F32)
            with tc.tile_pool(name="psL", bufs=2, space="PSUM") as psL:
                for cf in range(NBF):
                    pl = psL.tile([P, 512], F32, tag="labbc")
                    nc.tensor.matmul(
                        pl[:],
                        ones_row_f[:],
                        lab_row_t[0:1, cf * 512 : (cf + 1) * 512],
                        start=True,
                        stop=True,
                    )
                    nc.scalar.copy(lab_bc[:, cf * 512 : (cf + 1) * 512], pl[:])

            nc.gpsimd.collective_compute(
                "AllReduce",
                ALU.add,
                replica_groups=[list(range(N_CORES))],
                ins=[cc_in[:].opt()],
                outs=[cc_out[:].opt()],
            )

            # ---------------- phase C ----------------
            with tc.tile_pool(name="phC", bufs=1) as pc:
                segf8 = big.tile([P, ND, C], F8, name="segf8")
                for p4 in range(NPAIR):
                    eng = (nc.sync, nc.scalar, nc.gpsimd, nc.sync)[p4]
                    eng.dma_start(
                        segf8[:, 2 * p4 : 2 * p4 + 2, :],
                        cc_out[:, 2 * p4 : 2 * p4 + 2, :],
                    )
                # squares for |seg_c|^2 (same f8 values the Q matmul uses)
                sqC = []
                for d in range(ND):
                    sq = work.tile([P, C], BF16, tag="sqC", bufs=8, name=f"sqc{d}")
                    s8 = segf8[:, d, :]
                    nc.vector.tensor_tensor(sq[:], s8, s8, ALU.mult)
                    sqC.append(sq)

                ssq_row = constp.tile([1, C], F32)
                ssq_col = constp.tile([P, NCC], F32)
                bias_col = constp.tile([P, NCC], F32)
                st = constp.tile([P, NBLK + 1], F32)
                sp_dump = dump.tile([P, 512], F32, name="sp_dump")
                prod_dump = dump.tile([P, 512], F32, name="prod_dump")
                r_all = [pc.tile([P, 512], F32, name=f"r{k}") for k in range(NBLK)]
                ex_all = [pc.tile([P, 512], BF16, name=f"ex{k}") for k in range(NBLK)]

                def emit_sqrt_stt(blk, cc, bf, ps):
                    nc.scalar.activation(
                        r_all[blk][:],
                        ps[:],
                        AF.Sqrt,
                        bias=bias_col[:, cc : cc + 1],
                        scale=aux_t[:, 16 + cc : 17 + cc],
                    )
                    nc.vector.scalar_tensor_tensor(
                        prod_dump[:],
                        lab_bc[:, bf * 512 : (bf + 1) * 512],
                        aux_t[:, 8 + cc : 9 + cc],
                        r_all[blk][:],
                        op0=ALU.is_equal,
                        op1=ALU.mult,
                        accum_out=st[:, blk : blk + 1],
                    )

                with (
                    tc.tile_pool(name="psSim", bufs=4, space="PSUM") as psSim,
                    tc.tile_pool(name="psq", bufs=2, space="PSUM") as psq,
                    tc.tile_pool(name="psT", bufs=2, space="PSUM") as psT,
                ):
                    def emit_block_mms(blk):
                        cc, bf = blk // NBF, blk % NBF
                        ps = psSim.tile([P, 512], F32, tag="sim", name=f"sim{blk}")
                        for pb in range(NPAIR):
                            nc.tensor.matmul(
                                ps[:],
                                segf8[:, 2 * pb : 2 * pb + 2, cc * P : (cc + 1) * P],
                                zjt[:, 2 * pb : 2 * pb + 2, bf * 512 : (bf + 1) * 512],
                                start=(pb == 0),
                                stop=(pb == NPAIR - 1),
                                perf_mode=DR,
                            )
                        return cc, bf, ps

                    # held blocks double as PE pstate warm-up under bias prep
                    held = [(blk, *emit_block_mms(blk)) for blk in range(NHELD)]

                    # bias prep: ssq rows, transposes, folds
                    pq = [
                        psq.tile([1, 512], F32, tag="ssq", name=f"pq{i}")
                        for i in range(NCF)
                    ]
                    for cf in range(NCF):
                        for d in range(ND):
                            nc.tensor.matmul(
                                pq[cf][:],
                                ones_col_bf[:],
                                sqC[d][:, cf * 512 : (cf + 1) * 512],
                                start=(d == 0),
                                stop=(d == ND - 1),
                            )
                        nc.vector.tensor_copy(
                            ssq_row[0:1, cf * 512 : (cf + 1) * 512], pq[cf][:]
                        )
                    for cc in range(NCC):
                        pt = psT.tile([P, 1], F32, tag="tr")
                        nc.tensor.transpose(
                            pt[:], ssq_row[0:1, cc * P : (cc + 1) * P], ident1[:]
                        )
                        nc.vector.tensor_copy(ssq_col[:, cc : cc + 1], pt[:])
                    nc.vector.tensor_tensor(
                        bias_col[:], ssq_col[:], aux_t[:, 24:32], ALU.mult
                    )
                    nc.vector.tensor_scalar(
                        bias_col[:], bias_col[:], 1.0, None, ALU.add
                    )

                    def emit_exp(blk):
                        # softplus: exp (bias=2, scale=-1) then +1 on DVE
                        nc.scalar.activation(
                            ex_all[blk][:], r_all[blk][:], AF.Exp,
                            bias=two_col[:], scale=-1.0,
                        )
                        nc.vector.tensor_scalar(
                            ex_all[blk][:], ex_all[blk][:], 1.0, None, ALU.add
                        )

                    for blk, cc, bf, ps in held:
                        emit_sqrt_stt(blk, cc, bf, ps)
                    for blk in range(NHELD, NBLK):
                        cc, bf, ps = emit_block_mms(blk)
                        emit_sqrt_stt(blk, cc, bf, ps)
                    for k in range(NBLK):
                        emit_exp(k)

                step = 1
                while step < NBLK:
                    for k in range(0, NBLK, 2 * step):
                        nc.vector.tensor_tensor(
                            ex_all[k][:], ex_all[k][:], ex_all[k + step][:], ALU.mult
                        )
                    step *= 2
                nc.scalar.activation(
                    sp_dump[:],
                    ex_all[0][:],
                    AF.Ln,
                    accum_out=st[:, NBLK : NBLK + 1],
                )

                with tc.tile_pool(name="psFin", bufs=1, space="PSUM") as psFin:
                    pf = psFin.tile([1, NBLK + 1], F32, tag="fin")
                    nc.tensor.matmul(pf[:], ones_col_f[:], st[:], start=True, stop=True)
                    fin_row = constp.tile([1, NBLK + 1], F32)
                    nc.vector.tensor_copy(fin_row[:], pf[:])
                    tot = constp.tile([1, 1], F32)
                    nc.vector.tensor_reduce(tot[:], fin_row[:], axis=AX.X, op=ALU.add)
                    nc.sync.dma_start(out_partial[0:1, 0:1], tot[:])

    nc.compile()
    _NC_CACHE["nc"] = nc
    return nc


def make_in_maps(emb_i, emb_j, labels):
    emb_i = np.asarray(emb_i, dtype=np.float32)
    emb_j = np.asarray(emb_j, dtype=np.float32)
    lab = np.asarray(labels).astype(np.int64)
    labf = lab.astype(np.float32)
    counts = np.bincount(lab, minlength=C).astype(np.float32)  # all >= 1
    ccol = (
        np.arange(P, dtype=np.float32)[:, None]
        + P * np.arange(NCC, dtype=np.float32)[None, :]
    )
    cnt_col = counts.reshape(NCC, P).T          # [P, NCC], class cc*128+p
    scale_col = -2.0 / cnt_col
    ic2_col = 1.0 / (cnt_col * cnt_col)
    in_maps = []
    for k in range(N_CORES):
        sl = slice(k * BL, (k + 1) * BL)
        lab_k = lab[sl]
        # partition-major bf16 inputs
        eib = np.ascontiguousarray(
            emb_i[sl].astype(NP_BF16).reshape(NB, P, D).transpose(1, 0, 2)
        )
        ejtb = np.ascontiguousarray(
            emb_j[sl].T.astype(NP_BF16).reshape(ND, P, BL).transpose(1, 0, 2)
        )
        oh = np.zeros((BL, C), dtype=NP_F8)
        oh[np.arange(BL), lab_k] = NP_F8(1.0)
        ohr = np.ascontiguousarray(oh.reshape(NB, P, C).transpose(1, 0, 2))
        aux = np.zeros((P, 32), np.float32)
        aux[:, 8 : 8 + NCC] = ccol
        aux[:, 16 : 16 + NCC] = scale_col
        aux[:, 24 : 24 + NCC] = ic2_col
        in_maps.append(
            {
                "emb_ib": eib,
                "emb_jTb": ejtb,
                "oh_in": ohr,
                "aux": np.ascontiguousarray(aux),
                "lab_row": np.ascontiguousarray(labf[sl][None, :]),
            }
        )
    return in_maps


def combine_partials(results):
    tot = 0.0
    for k in range(N_CORES):
        p = np.asarray(results[k]["out_partial"], dtype=np.float64)
        tot += p[0, 0]
    loss = (tot - 2.0 * B) / (B * C)
    return np.asarray(np.float32(loss))


def run(emb_i, emb_j, labels, **run_kwargs):
    nc = build_nc()
    in_maps = make_in_maps(emb_i, emb_j, labels)
    res = bass_utils.run_bass_kernel_spmd(
        nc, in_maps, core_ids=list(range(N_CORES)), **run_kwargs
    )
    return combine_partials(res.results), res


def kernel(emb_i, emb_j, labels):
    loss, _ = run(emb_i, emb_j, labels)
    return loss
